# revision 1
# baseline (speedup 1.0000x reference)
# GGNN encoder kernel for Trainium2 (Bass/Tile), data-parallel over the
# batch dimension: 8 graphs -> 8 NeuronCores, one graph per core.
#
# Per-core computation (one graph):
#   type_e  = type_table[node_types]                       # [N, TD]
#   tok_e   = word_emb[node_token_ids]                     # [T, D]   (SWDGE dma_gather)
#   text_e  = segment_mean(tok_e, token_seg_ids)           # [N, D]   (PE matmul w/ pooling matrix)
#   h       = concat(type_e, text_e) @ fusion_w + b        # [N, D]
#   4 x GGNN layer:
#     m    = h @ Wl                                        # [N, D]
#     agg  = A @ m          (A dense adjacency, built host-side from edge list)
#     GRU(h, agg)
#   out     = mask * h
#
# Layout strategy: h, agg, gates are kept feature-major ("T" layout,
# [feat partitions, node free-dim]) so that the feature-contracting GRU
# matmuls can run directly; m is node-major for the node-contracting
# scatter matmul. Matmuls run as float32r (full fp32 storage, single-pass
# PE mode) for 4x throughput over plain fp32.

import functools

import numpy as np

import concourse.bass as bass
import concourse.mybir as mybir
import concourse.tile as tile
from concourse import bacc, bass_utils
from concourse.masks import make_identity

# Problem shapes (hardcoded: kernel must be self-contained).
B, N, T, D, TD, L = 8, 512, 2048, 768, 128, 4
V, TYPES = 30522, 64
MAX_NODE_LEN = 512
K3 = 3 * D            # 2304 stacked GRU gate rows
F = TD + D            # 896 fused embedding dim
P = 128               # partitions
NCH = N // P          # 4 node chunks
TCH = T // P          # 16 token chunks
DCH = D // P          # 6 feature chunks
FCH = F // P          # 7 fused-dim chunks
GCH = 3 * DCH         # 18 gate row chunks
BLK = N // TCH        # 32 nodes per token chunk (block-pooling case)
NF = 512              # free-dim tile (nodes)
GS = 4                # token gather splits
GT = T // GS          # tokens per gather split (512)
GC = GT // P          # 128-chunks per gather split (4)

f32 = mybir.dt.float32
f32r = mybir.dt.float32r
i32 = mybir.dt.int32
i16 = mybir.dt.int16

Sigmoid = mybir.ActivationFunctionType.Sigmoid
Tanh = mybir.ActivationFunctionType.Tanh
Ident = mybir.ActivationFunctionType.Identity


def build_nc(pool_wide: bool) -> bass.Bass:
    nc = bacc.Bacc(num_swdge_queues=2, dynamic_dma_scratch_size=32768)

    # All host-side tensors are pre-laid-out partition-major so every DMA is
    # contiguous per partition.
    tok_idx = nc.dram_tensor("tok_idx", [P, GS * (GT // 16)], i16,
                             kind="ExternalInput")  # [128, 4*32] wrapped idxs
    typ_oh = nc.dram_tensor("typ_oh", [TYPES, N], f32r, kind="ExternalInput")
    word_emb = nc.dram_tensor("word_emb", [V, D], f32r, kind="ExternalInput")
    type_table = nc.dram_tensor("type_table", [TYPES, TD], f32r, kind="ExternalInput")
    pool_w = N if pool_wide else BLK
    poolm = nc.dram_tensor("poolm", [P, TCH, pool_w], f32r, kind="ExternalInput")
    at_w = nc.dram_tensor("at_w", [P, NCH, N], f32r, kind="ExternalInput")
    fusion_w = nc.dram_tensor("fusion_w", [F, D], f32r, kind="ExternalInput")
    fusion_b = nc.dram_tensor("fusion_b", [P, DCH], f32, kind="ExternalInput")
    wl = nc.dram_tensor("wl", [L, DCH, P, D], f32r, kind="ExternalInput")
    wih = nc.dram_tensor("wih", [P, DCH, K3], f32r, kind="ExternalInput")
    whh_st = nc.dram_tensor("whh_st", [GCH, P, DCH, P], f32r, kind="ExternalInput")
    bsum = nc.dram_tensor("bsum", [P, GCH], f32, kind="ExternalInput")
    bihn = nc.dram_tensor("bihn", [P, DCH], f32, kind="ExternalInput")
    bhhn = nc.dram_tensor("bhhn", [P, DCH], f32, kind="ExternalInput")
    maskc = nc.dram_tensor("maskc", [P, NCH], f32, kind="ExternalInput")
    out = nc.dram_tensor("out", [N, D], f32, kind="ExternalOutput")

    with tile.TileContext(nc) as tc:
        with (
            tc.tile_pool(name="consts", bufs=1) as consts,
            tc.tile_pool(name="wbig", bufs=1) as wbig,
            tc.tile_pool(name="t768", bufs=7) as t768,
            tc.tile_pool(name="c512", bufs=7) as c512,
            tc.tile_pool(name="hpool", bufs=12) as hpool,
            tc.tile_pool(name="gpool", bufs=5) as gpool,
            tc.tile_pool(name="wst", bufs=3) as wst,
            tc.tile_pool(name="wlc", bufs=7) as wlc,
            tc.tile_pool(name="tokg", bufs=2) as tokg,
            tc.tile_pool(name="psA", bufs=7, space="PSUM") as psA,
        ):
            # ---- token gather first: it gates the whole front of the kernel
            tok_idx_sb = consts.tile([P, T // 16], i16)
            nc.sync.dma_start(out=tok_idx_sb[:], in_=tok_idx[:])
            pool_sb = consts.tile([P, TCH, pool_w], f32r)
            nc.sync.dma_start(out=pool_sb[:], in_=poolm[:])

            # type embeddings via one-hot matmul: two tiny DMAs + one PE op,
            # nothing queues behind the big token gathers
            tt_sb = consts.tile([TYPES, TD], f32r)
            nc.sync.dma_start(out=tt_sb[:], in_=type_table[:])
            oh_sb = consts.tile([TYPES, N], f32r)
            nc.sync.dma_start(out=oh_sb[:], in_=typ_oh[:])

            gath = []
            gath_insts = []
            for s in range(GS):
                tg = tokg.tile([P, GC, D], f32r, tag="tokg", name=f"tokg{s}")
                gi_ = nc.gpsimd.dma_gather(
                    tg[:],
                    word_emb[:],
                    tok_idx_sb[:, s * (GT // 16) : (s + 1) * (GT // 16)],
                    GT,
                    GT,
                    D,
                    queue_num=s % 2,
                )
                gath.append(tg)
                gath_insts.append(gi_)

            def after_gathers(dma_inst):
                return dma_inst

            # ---- remaining constants / small inputs ----
            identity = consts.tile([P, P], f32)
            make_identity(nc, identity[:])
            bsum_sb = consts.tile([P, GCH], f32)
            nc.sync.dma_start(out=bsum_sb[:], in_=bsum[:])
            bihn_sb = consts.tile([P, DCH], f32)
            nc.sync.dma_start(out=bihn_sb[:], in_=bihn[:])
            bhhn_sb = consts.tile([P, DCH], f32)
            nc.sync.dma_start(out=bhhn_sb[:], in_=bhhn[:])
            fb_sb = consts.tile([P, DCH], f32)
            nc.sync.dma_start(out=fb_sb[:], in_=fusion_b[:])
            mask_sb = consts.tile([P, NCH], f32)
            nc.sync.dma_start(out=mask_sb[:], in_=maskc[:])

            # ---- fused embedding (feature-major [f, n]) ----
            fusedT = [
                c512.tile([P, NF], f32r, tag="c512", name=f"fusedT{k}")
                for k in range(FCH)
            ]

            # weight loads, emitted in the order the compute will need them
            # (the DMA engines drain roughly in emission order)
            fw = []
            for k in range(FCH):
                fwk = t768.tile([P, D], f32r, tag="t768", name=f"fw{k}")
                after_gathers(nc.scalar.dma_start(
                    out=fwk[:], in_=fusion_w[k * P : (k + 1) * P, :]
                ))
                fw.append(fwk)
            wlk = []
            for k in range(DCH):
                wk = wlc.tile([P, D], f32r, tag="wlc", name=f"wl0_{k}")
                after_gathers(nc.scalar.dma_start(out=wk[:], in_=wl[0, k]))
                wlk.append(wk)
            at_sb = wbig.tile([P, NCH, N], f32r)
            after_gathers(nc.scalar.dma_start(out=at_sb[:], in_=at_w[:]))
            wih_sb = wbig.tile([P, DCH, K3], f32r)

            # type_eT = type_table.T @ onehot  (one matmul, K=64)
            ptyp = psA.tile([P, NF], f32, tag="psA")
            nc.tensor.matmul(
                out=ptyp[:], lhsT=tt_sb[:], rhs=oh_sb[:], start=True, stop=True
            )
            nc.vector.tensor_copy(out=fusedT[0][:], in_=ptyp[:])

            # token pooling: PE matmul pools 128 tokens -> 32 nodes and
            # transposes to feature-major in one pass
            for s in range(GS):
                tg = gath[s]
                for c2 in range(GC):
                    c = s * GC + c2
                    if pool_wide:
                        for f in range(DCH):
                            pc = psA.tile([P, NF], f32, tag="psA")
                            nc.tensor.matmul(
                                out=pc[:],
                                lhsT=tg[:, c2, f * P : (f + 1) * P],
                                rhs=pool_sb[:, c, :],
                                start=True,
                                stop=True,
                            )
                            if c == 0:
                                nc.vector.tensor_copy(out=fusedT[1 + f][:], in_=pc[:])
                            else:
                                nc.vector.tensor_add(
                                    out=fusedT[1 + f][:],
                                    in0=fusedT[1 + f][:],
                                    in1=pc[:],
                                )
                    else:
                        pc = psA.tile([P, DCH * BLK], f32, tag="psA")
                        for f in range(DCH):
                            nc.tensor.matmul(
                                out=pc[:, f * BLK : (f + 1) * BLK],
                                lhsT=tg[:, c2, f * P : (f + 1) * P],
                                rhs=pool_sb[:, c, :],
                                start=True,
                                stop=True,
                            )
                        for f in range(DCH):
                            nc.vector.tensor_copy(
                                out=fusedT[1 + f][:, c * BLK : (c + 1) * BLK],
                                in_=pc[:, f * BLK : (f + 1) * BLK],
                            )

            # ---- fusion matmul: hT[j] = (fusion_w.T @ fusedT)[j] + b ----
            hT = []
            for j in range(DCH):
                pf = psA.tile([P, NF], f32, tag="psA")
                for k in range(FCH):
                    nc.tensor.matmul(
                        out=pf[:],
                        lhsT=fw[k][:, j * P : (j + 1) * P],
                        rhs=fusedT[k][:],
                        start=(k == 0),
                        stop=(k == FCH - 1),
                    )
                hj = hpool.tile([P, NF], f32r, tag="hpool")
                nc.scalar.activation(
                    out=hj[:], in_=pf[:], func=Ident, bias=fb_sb[:, j : j + 1]
                )
                hT.append(hj)
                after_gathers(nc.scalar.dma_start(out=wih_sb[:, j, :], in_=wih[:, j, :]))

            # ---- GGNN layers ----
            for l in range(L):
                # m = h @ Wl   (node-major out, [node 128, 768] per chunk)
                if l > 0:
                    wlk = []
                    for k in range(DCH):
                        wk = wlc.tile([P, D], f32r, tag="wlc", name=f"wl{l}_{k}")
                        nc.scalar.dma_start(out=wk[:], in_=wl[l, k])
                        wlk.append(wk)
                m_sb = []
                for i in range(NCH):
                    pma = psA.tile([P, NF], f32, tag="psA")
                    pmb = psA.tile([P, D - NF], f32, tag="psA")
                    for k in range(DCH):
                        nc.tensor.matmul(
                            out=pma[:],
                            lhsT=hT[k][:, i * P : (i + 1) * P],
                            rhs=wlk[k][:, :NF],
                            start=(k == 0),
                            stop=(k == DCH - 1),
                        )
                        nc.tensor.matmul(
                            out=pmb[:],
                            lhsT=hT[k][:, i * P : (i + 1) * P],
                            rhs=wlk[k][:, NF:D],
                            start=(k == 0),
                            stop=(k == DCH - 1),
                        )
                    mi = t768.tile([P, D], f32r, tag="t768", name=f"m{l}_{i}")
                    nc.vector.tensor_copy(out=mi[:, :NF], in_=pma[:])
                    nc.vector.tensor_copy(out=mi[:, NF:D], in_=pmb[:])
                    m_sb.append(mi)

                # aggT = m.T @ A.T  (feature-major [feat 128, nodes 512])
                aggT = []
                for j in range(DCH):
                    pa = psA.tile([P, NF], f32, tag="psA")
                    for k in range(NCH):
                        nc.tensor.matmul(
                            out=pa[:],
                            lhsT=m_sb[k][:, j * P : (j + 1) * P],
                            rhs=at_sb[:, k, :],
                            start=(k == 0),
                            stop=(k == NCH - 1),
                        )
                    aj = c512.tile([P, NF], f32r, tag="c512", name=f"agg{l}_{j}")
                    nc.vector.tensor_copy(out=aj[:], in_=pa[:])
                    aggT.append(aj)

                # GRU gates, 128 gate rows at a time
                hnew = []
                for i in range(DCH):
                    # streamed Whh chunks for the three gates at row-chunk i
                    wch = []
                    for g in range(3):
                        w = wst.tile([P, DCH, P], f32r, tag="wst",
                                     name=f"wch{l}_{i}_{g}")
                        wdma = nc.sync.dma_start(out=w[:], in_=whh_st[g * DCH + i])
                        if l == 0 and i == 0:
                            after_gathers(wdma)
                        wch.append(w)

                    # r and z: psum accumulates gi + gh, ACT adds bias+sigmoid
                    rz = []
                    for g in range(2):
                        pg = psA.tile([P, NF], f32, tag="psA")
                        col = g * D + i * P
                        # gh first: it only needs h + the small whh stream,
                        # so it runs while wih/aggT are still in flight
                        for k in range(DCH):
                            nc.tensor.matmul(
                                out=pg[:],
                                lhsT=wch[g][:, k, :],
                                rhs=hT[k][:],
                                start=(k == 0),
                                stop=False,
                            )
                        for k in range(DCH):
                            nc.tensor.matmul(
                                out=pg[:],
                                lhsT=wih_sb[:, k, col : col + P],
                                rhs=aggT[k][:],
                                start=False,
                                stop=(k == DCH - 1),
                            )
                        gs = gpool.tile([P, NF], f32, tag="gpool",
                                        name=f"g{l}_{i}_{g}")
                        nc.scalar.activation(
                            out=gs[:],
                            in_=pg[:],
                            func=Sigmoid,
                            bias=bsum_sb[:, g * DCH + i : g * DCH + i + 1],
                        )
                        rz.append(gs)
                    r_sb, z_sb = rz

                    # n gate: keep gi and gh separate
                    col = 2 * D + i * P
                    pghn = psA.tile([P, NF], f32, tag="psA")
                    for k in range(DCH):
                        nc.tensor.matmul(
                            out=pghn[:],
                            lhsT=wch[2][:, k, :],
                            rhs=hT[k][:],
                            start=(k == 0),
                            stop=(k == DCH - 1),
                        )
                    pgin = psA.tile([P, NF], f32, tag="psA")
                    for k in range(DCH):
                        nc.tensor.matmul(
                            out=pgin[:],
                            lhsT=wih_sb[:, k, col : col + P],
                            rhs=aggT[k][:],
                            start=(k == 0),
                            stop=(k == DCH - 1),
                        )
                    hb = gpool.tile([P, NF], f32, tag="gpool")
                    nc.scalar.activation(
                        out=hb[:], in_=pghn[:], func=Ident,
                        bias=bhhn_sb[:, i : i + 1],
                    )
                    rn = gpool.tile([P, NF], f32, tag="gpool")
                    nc.vector.tensor_mul(out=rn[:], in0=r_sb[:], in1=hb[:])
                    tn = gpool.tile([P, NF], f32, tag="gpool")
                    nc.vector.tensor_add(out=tn[:], in0=pgin[:], in1=rn[:])
                    nn_ = gpool.tile([P, NF], f32, tag="gpool")
                    nc.scalar.activation(
                        out=nn_[:], in_=tn[:], func=Tanh,
                        bias=bihn_sb[:, i : i + 1],
                    )
                    # h' = n + z * (h - n)
                    s_ = gpool.tile([P, NF], f32, tag="gpool")
                    nc.vector.tensor_sub(out=s_[:], in0=hT[i][:], in1=nn_[:])
                    sz = gpool.tile([P, NF], f32, tag="gpool")
                    nc.vector.tensor_mul(out=sz[:], in0=z_sb[:], in1=s_[:])
                    hj = hpool.tile([P, NF], f32r, tag="hpool",
                                    name=f"h{l}_{i}")
                    nc.vector.tensor_add(out=hj[:], in0=nn_[:], in1=sz[:])
                    hnew.append(hj)
                hT = hnew

            # ---- transpose back to node-major, mask, write out ----
            for i in range(NCH):
                poa = psA.tile([P, NF], f32, tag="psA")
                pob = psA.tile([P, D - NF], f32, tag="psA")
                for j in range(DCH):
                    dst = poa[:, j * P : (j + 1) * P] if j < 4 else \
                        pob[:, (j - 4) * P : (j - 3) * P]
                    nc.tensor.transpose(
                        out=dst,
                        in_=hT[j][:, i * P : (i + 1) * P].bitcast(f32),
                        identity=identity[:],
                    )
                ob = t768.tile([P, D], f32, tag="t768")
                nc.vector.tensor_scalar_mul(
                    out=ob[:, :NF], in0=poa[:], scalar1=mask_sb[:, i : i + 1]
                )
                nc.vector.tensor_scalar_mul(
                    out=ob[:, NF:D], in0=pob[:], scalar1=mask_sb[:, i : i + 1]
                )
                nc.sync.dma_start(out=out[i * P : (i + 1) * P, :], in_=ob[:])

    nc.compile()
    return nc


@functools.lru_cache(maxsize=2)
def _get_nc(pool_wide: bool) -> bass.Bass:
    return build_nc(pool_wide)


def _prep_shared(inputs):
    """Weight tensors identical across graphs, pre-laid-out partition-major."""
    fusion_w = np.ascontiguousarray(np.asarray(inputs["fusion_w"], np.float32))
    fusion_b = np.ascontiguousarray(
        np.asarray(inputs["fusion_b"], np.float32).reshape(DCH, P).T
    )
    wl = np.ascontiguousarray(
        np.asarray(inputs["ggnn_w"], np.float32).reshape(L, DCH, P, D)
    )
    wih_w = np.asarray(inputs["gru_w_ih"], np.float32)   # [K3, D]
    whh_w = np.asarray(inputs["gru_w_hh"], np.float32)
    bih = np.asarray(inputs["gru_b_ih"], np.float32)
    bhh = np.asarray(inputs["gru_b_hh"], np.float32)
    # wih: [P, DCH, K3]  (partition p, feat chunk k -> gate rows)
    wihT = wih_w.T                                       # [D, K3]
    wih = np.ascontiguousarray(wihT.reshape(DCH, P, K3).transpose(1, 0, 2))
    # whh chunks: [GCH, P, DCH, P]
    whhT = whh_w.T                                       # [D, K3]
    whh_st = np.ascontiguousarray(
        np.stack(
            [
                whhT[:, j * P : (j + 1) * P].reshape(DCH, P, P).transpose(1, 0, 2)
                for j in range(GCH)
            ]
        )
    )
    bsum = np.ascontiguousarray((bih + bhh).reshape(GCH, P).T)
    bihn = np.ascontiguousarray(bih[2 * D :].reshape(DCH, P).T)
    bhhn = np.ascontiguousarray(bhh[2 * D :].reshape(DCH, P).T)
    word_emb = np.ascontiguousarray(np.asarray(inputs["word_emb"], np.float32))
    type_table = np.ascontiguousarray(np.asarray(inputs["type_table"], np.float32))
    return dict(
        word_emb=word_emb, type_table=type_table, fusion_w=fusion_w,
        fusion_b=fusion_b, wl=wl, wih=wih, whh_st=whh_st, bsum=bsum,
        bihn=bihn, bhhn=bhhn,
    )


def _graph_blockable(inputs, b):
    seg = np.asarray(inputs["token_seg_ids"][b], np.int64)
    tcol = np.arange(T) // P
    return bool(np.all((seg >= tcol * BLK) & (seg < (tcol + 1) * BLK)))


def _prep_graph(inputs, b, pool_wide):
    tok = np.asarray(inputs["node_token_ids"][b], np.int64)
    typ = np.asarray(inputs["node_types"][b], np.int32)
    seg = np.asarray(inputs["token_seg_ids"][b], np.int64)
    lens = np.asarray(inputs["node_token_lens"][b], np.float64)
    glen = int(np.asarray(inputs["graph_node_lens"][b]))
    esrc = np.asarray(inputs["edge_src"][b], np.int64)
    edst = np.asarray(inputs["edge_dst"][b], np.int64)
    ew = np.asarray(inputs["edge_weight"][b], np.float32)

    # token idxs for dma_gather: GS splits of GT idxs, each wrapped into
    # 16 partitions ([p, s] = idx[s*16+p]) and replicated to 128 partitions
    tok16 = tok.astype(np.int16)
    cols = []
    for s in range(GS):
        w16 = tok16[s * GT : (s + 1) * GT].reshape(GT // 16, 16).T  # [16, GT/16]
        cols.append(np.tile(w16, (8, 1)))                           # [128, GT/16]
    tok_idx = np.ascontiguousarray(np.concatenate(cols, axis=1))    # [128, GS*32]

    typ_oh = np.zeros((TYPES, N), np.float32)
    typ_oh[typ, np.arange(N)] = 1.0

    # dense transposed adjacency: AT[src, dst], laid out [P, NCH, N]
    at = np.zeros((N, N), np.float32)
    np.add.at(at, (esrc, edst), ew)
    at = np.ascontiguousarray(at.reshape(NCH, P, N).transpose(1, 0, 2))

    # pooling matrix (1/len weights), [P, TCH, BLK or N]
    winv = np.zeros(N, np.float64)
    nzmask = lens != 0
    winv[nzmask] = 1.0 / lens[nzmask]
    tcol = np.arange(T) // P  # token chunk of each token
    if pool_wide:
        poolm = np.zeros((TCH, P, N), np.float32)
        poolm[tcol, np.arange(T) % P, seg] = winv[seg]
    else:
        poolm = np.zeros((TCH, P, BLK), np.float32)
        poolm[tcol, np.arange(T) % P, seg - tcol * BLK] = winv[seg]
    poolm = np.ascontiguousarray(poolm.transpose(1, 0, 2))

    keep = min(glen, MAX_NODE_LEN)
    mask = np.ascontiguousarray(
        (np.arange(N) < keep).astype(np.float32).reshape(NCH, P).T
    )
    return dict(tok_idx=tok_idx, typ_oh=typ_oh, at_w=at, poolm=poolm,
                maskc=mask)


def kernel(**inputs) -> np.ndarray:
    shared = _prep_shared(inputs)
    pool_wide = not all(_graph_blockable(inputs, b) for b in range(B))
    per_graph = [_prep_graph(inputs, b, pool_wide) for b in range(B)]
    nc = _get_nc(pool_wide)
    in_maps = [{**shared, **per_graph[b]} for b in range(B)]
    res = bass_utils.run_bass_kernel_spmd(nc, in_maps, core_ids=list(range(B)))
    global _last_exec_ns
    _last_exec_ns = res.exec_time_ns
    out = np.stack([r["out"] for r in res.results]).astype(np.float32)
    return out


_last_exec_ns = None



# revision 2
# speedup vs baseline: 1.9225x; 1.9225x over previous
# GGNN encoder kernel for Trainium2 (Bass/Tile), data-parallel over the
# batch dimension: 8 graphs -> 8 NeuronCores, one graph per core.
#
# Per-core computation (one graph):
#   type_e  = type_table[node_types]                       # [N, TD]
#   tok_e   = word_emb[node_token_ids]                     # [T, D]   (SWDGE dma_gather)
#   text_e  = segment_mean(tok_e, token_seg_ids)           # [N, D]   (PE matmul w/ pooling matrix)
#   h       = concat(type_e, text_e) @ fusion_w + b        # [N, D]
#   4 x GGNN layer (algebraically refactored):
#     ah   = A @ h               (A dense adjacency; replaces A @ (h@Wl))
#     gi   = ah @ Wc[l]          (Wc[l] = Wl[l] @ W_ih^T, host-precomputed)
#     gh   = h @ W_hh^T
#     GRU(h, gi, gh)
#   out     = mask * h
#
# Precision strategy (validated against the reference numerics):
#   - r/z gate GEMMs and gh_n run in fp8-e4m3 with DoubleRow perf mode
#     (2 MACs/cell/cycle, K=256 per matmul). Operands are pre-scaled by
#     powers of two with matching products (SH*SWH == SA*SWC == 1024) so
#     both GEMMs accumulate into one PSUM group; the 1/1024 descale folds
#     into the activation's scale input.
#   - gi_n (the GRU candidate-gate input transform) is error-critical and
#     stays fp16; A@h, fusion, pooling and embeddings are fp16 as well.
#   - All PSUM accumulation is fp32; GRU elementwise runs in fp16.

import functools

import numpy as np
import ml_dtypes

import concourse.bass as bass
import concourse.mybir as mybir
import concourse.tile as tile
from concourse import bacc, bass_utils
from concourse.masks import make_identity

# Problem shapes (hardcoded: kernel must be self-contained).
B, N, T, D, TD, L = 8, 512, 2048, 768, 128, 4
V, TYPES = 30522, 64
MAX_NODE_LEN = 512
K3 = 3 * D            # 2304 stacked GRU gate rows
F = TD + D            # 896 fused embedding dim
P = 128               # partitions
NCH = N // P          # 4 node chunks
TCH = T // P          # 16 token chunks
DCH = D // P          # 6 feature chunks
FCH = F // P          # 7 fused-dim chunks
GCH = 3 * DCH         # 18 gate row chunks
RZCH = 2 * DCH        # 12 r/z gate row chunks
KP = DCH // 2         # 3 contraction k-pairs for DoubleRow
BLK = N // TCH        # 32 nodes per token chunk (block-pooling case)
GS = 4                # token gather splits
GT = T // GS          # tokens per gather split (512)
GC = GT // P          # 128-chunks per gather split (4)

SH = 64.0             # fp8 scale on h
SWH = 16.0            # fp8 scale on W_hh^T
SA = 16.0             # fp8 scale on ah
SWC = 64.0            # fp8 scale on Wc_rz
SINV = 1.0 / (SH * SWH)   # descale (== 1/(SA*SWC))

f32 = mybir.dt.float32
f16 = mybir.dt.float16
f8 = mybir.dt.float8e4
i16 = mybir.dt.int16
DR = mybir.MatmulPerfMode.DoubleRow

Sigmoid = mybir.ActivationFunctionType.Sigmoid
Tanh = mybir.ActivationFunctionType.Tanh
Ident = mybir.ActivationFunctionType.Identity


def build_nc(pool_wide: bool) -> bass.Bass:
    nc = bacc.Bacc(num_swdge_queues=2, dynamic_dma_scratch_size=32768)

    # All host-side tensors are pre-laid-out partition-major so every DMA is
    # contiguous per partition.
    tok_idx = nc.dram_tensor("tok_idx", [P, GS * (GT // 16)], i16,
                             kind="ExternalInput")  # [128, 4*32] wrapped idxs
    typ_oh = nc.dram_tensor("typ_oh", [TYPES, N], f16, kind="ExternalInput")
    word_emb = nc.dram_tensor("word_emb", [V, D], f16, kind="ExternalInput")
    type_table = nc.dram_tensor("type_table", [TYPES, TD], f16, kind="ExternalInput")
    pool_w = N if pool_wide else BLK
    poolm = nc.dram_tensor("poolm", [P, TCH, pool_w], f16, kind="ExternalInput")
    at_w = nc.dram_tensor("at_w", [P, NCH, N], f16, kind="ExternalInput")
    fusion_w = nc.dram_tensor("fusion_w", [F, D], f16, kind="ExternalInput")
    fusion_b = nc.dram_tensor("fusion_b", [P, DCH], f32, kind="ExternalInput")
    fusion_b64 = nc.dram_tensor("fusion_b64", [P, DCH], f32, kind="ExternalInput")
    wc8 = nc.dram_tensor("wc8", [L, P, RZCH, KP, 2, P], f8, kind="ExternalInput")
    wcn = nc.dram_tensor("wcn", [L, P, DCH, DCH, P], f16, kind="ExternalInput")
    whh8 = nc.dram_tensor("whh8", [P, GCH, KP, 2, P], f8, kind="ExternalInput")
    bsum = nc.dram_tensor("bsum", [P, GCH], f32, kind="ExternalInput")
    bihn = nc.dram_tensor("bihn", [P, DCH], f32, kind="ExternalInput")
    bhhn = nc.dram_tensor("bhhn", [P, DCH], f32, kind="ExternalInput")
    maskc = nc.dram_tensor("maskc", [P, NCH], f32, kind="ExternalInput")
    out = nc.dram_tensor("out", [N, D], f32, kind="ExternalOutput")

    with tile.TileContext(nc) as tc:
        with (
            tc.tile_pool(name="consts", bufs=1) as consts,
            tc.tile_pool(name="wres", bufs=1) as wres,
            tc.tile_pool(name="wstr", bufs=2) as wstr,
            tc.tile_pool(name="hpool", bufs=12) as hpool,
            tc.tile_pool(name="h8pool", bufs=6) as h8pool,
            tc.tile_pool(name="hnpool", bufs=8) as hnpool,
            tc.tile_pool(name="apool", bufs=8) as apool,
            tc.tile_pool(name="gpool", bufs=6) as gpool,
            tc.tile_pool(name="tokg", bufs=2) as tokg,
            tc.tile_pool(name="opool", bufs=2) as opool,
            tc.tile_pool(name="psT", bufs=2, space="PSUM") as psTp,
            tc.tile_pool(name="psRZ", bufs=3, space="PSUM") as psRZp,
            tc.tile_pool(name="psM", bufs=3, space="PSUM") as psMp,
        ):
            # ---- token gather first: it gates the whole front of the kernel
            tok_idx_sb = consts.tile([P, T // 16], i16)
            nc.sync.dma_start(out=tok_idx_sb[:], in_=tok_idx[:])
            pool_sb = consts.tile([P, TCH, pool_w], f16)
            nc.sync.dma_start(out=pool_sb[:], in_=poolm[:])

            # type embeddings via one-hot matmul: two tiny DMAs + one PE op
            tt_sb = consts.tile([TYPES, TD], f16)
            nc.scalar.dma_start(out=tt_sb[:], in_=type_table[:])
            oh_sb = consts.tile([TYPES, N], f16)
            nc.scalar.dma_start(out=oh_sb[:], in_=typ_oh[:])

            gath = []
            for s in range(GS):
                tg = tokg.tile([P, GC, D], f16, tag="tokg", name=f"tokg{s}")
                nc.gpsimd.dma_gather(
                    tg[:],
                    word_emb[:],
                    tok_idx_sb[:, s * (GT // 16) : (s + 1) * (GT // 16)],
                    GT,
                    GT,
                    D,
                    queue_num=s % 2,
                )
                gath.append(tg)

            # ---- remaining constants / small inputs ----
            identity = consts.tile([P, P], f16)
            make_identity(nc, identity[:])
            bsum_sb = consts.tile([P, GCH], f32)
            nc.sync.dma_start(out=bsum_sb[:], in_=bsum[:])
            bihn_sb = consts.tile([P, DCH], f32)
            nc.sync.dma_start(out=bihn_sb[:], in_=bihn[:])
            bhhn_sb = consts.tile([P, DCH], f32)
            nc.sync.dma_start(out=bhhn_sb[:], in_=bhhn[:])
            fb_sb = consts.tile([P, DCH], f32)
            nc.sync.dma_start(out=fb_sb[:], in_=fusion_b[:])
            fb64_sb = consts.tile([P, DCH], f32)
            nc.sync.dma_start(out=fb64_sb[:], in_=fusion_b64[:])
            mask_sb = consts.tile([P, NCH], f32)
            nc.sync.dma_start(out=mask_sb[:], in_=maskc[:])

            # fusion weights (needed ~6us in)
            fw = []
            for k in range(FCH):
                fwk = consts.tile([P, D], f16, name=f"fw{k}")
                nc.scalar.dma_start(out=fwk[:], in_=fusion_w[k * P : (k + 1) * P, :])
                fw.append(fwk)

            # adjacency + resident GRU hidden weights
            at_sb = wres.tile([P, NCH, N], f16)
            nc.sync.dma_start(out=at_sb[:], in_=at_w[:])
            whh8_sb = wres.tile([P, GCH, KP, 2, P], f8)
            nc.sync.dma_start(out=whh8_sb[:], in_=whh8[:])

            # layer-0 streamed weights
            def load_layer_w(l):
                wc8_l = wstr.tile([P, RZCH, KP, 2, P], f8, tag="wc8",
                                  name=f"wc8_{l}")
                nc.sync.dma_start(out=wc8_l[:], in_=wc8[l])
                wcn_l = wstr.tile([P, DCH, DCH, P], f16, tag="wcn",
                                  name=f"wcn_{l}")
                nc.sync.dma_start(out=wcn_l[:], in_=wcn[l])
                return wc8_l, wcn_l

            next_w = load_layer_w(0)

            # ---- fused embedding (feature-major [f, n]), single f16 tile ----
            fusedT = consts.tile([P, FCH, N], f16)

            # type_eT = type_table.T @ onehot  (one matmul, K=64)
            ptyp = psMp.tile([P, N], f32, tag="psM")
            nc.tensor.matmul(
                out=ptyp[:], lhsT=tt_sb[:], rhs=oh_sb[:], start=True, stop=True
            )
            nc.vector.tensor_copy(out=fusedT[:, 0, :], in_=ptyp[:])

            # token pooling: PE matmul pools 128 tokens -> 32 nodes and
            # transposes to feature-major in one pass
            for s in range(GS):
                tg = gath[s]
                for c2 in range(GC):
                    c = s * GC + c2
                    if pool_wide:
                        for fi in range(DCH):
                            pc = psMp.tile([P, N], f32, tag="psM")
                            nc.tensor.matmul(
                                out=pc[:],
                                lhsT=tg[:, c2, fi * P : (fi + 1) * P],
                                rhs=pool_sb[:, c, :],
                                start=True,
                                stop=True,
                            )
                            if c == 0:
                                nc.vector.tensor_copy(
                                    out=fusedT[:, 1 + fi, :], in_=pc[:]
                                )
                            else:
                                nc.vector.tensor_add(
                                    out=fusedT[:, 1 + fi, :],
                                    in0=fusedT[:, 1 + fi, :],
                                    in1=pc[:],
                                )
                    else:
                        pc = psMp.tile([P, DCH, BLK], f32, tag="psM")
                        for fi in range(DCH):
                            nc.tensor.matmul(
                                out=pc[:, fi, :],
                                lhsT=tg[:, c2, fi * P : (fi + 1) * P],
                                rhs=pool_sb[:, c, :],
                                start=True,
                                stop=True,
                            )
                        nc.vector.tensor_copy(
                            out=fusedT[:, 1 : 1 + DCH, c * BLK : (c + 1) * BLK],
                            in_=pc[:],
                        )

            # ---- fusion matmul -> hT0 (f16) + hq8_0 (fp8, x64) ----
            hT = []
            hq8 = []
            for j in range(DCH):
                pf = psMp.tile([P, N], f32, tag="psM")
                for k in range(FCH):
                    nc.tensor.matmul(
                        out=pf[:],
                        lhsT=fw[k][:, j * P : (j + 1) * P],
                        rhs=fusedT[:, k, :],
                        start=(k == 0),
                        stop=(k == FCH - 1),
                    )
                hj = hpool.tile([P, N], f16, tag="hT", name=f"h0_{j}")
                nc.scalar.activation(
                    out=hj[:], in_=pf[:], func=Ident, bias=fb_sb[:, j : j + 1]
                )
                hT.append(hj)
                if j % 2 == 0:
                    h8 = h8pool.tile([P, 2, N], f8, tag="hq8", name=f"hq8_0_{j // 2}")
                    hq8.append(h8)
                nc.scalar.activation(
                    out=hq8[j // 2][:, j % 2, :], in_=pf[:], func=Ident,
                    bias=fb64_sb[:, j : j + 1], scale=SH,
                )

            # ---- GGNN layers ----
            for l in range(L):
                wc8_l, wcn_l = next_w
                if l + 1 < L:
                    next_w = load_layer_w(l + 1)

                # node-major h (f16) via PE transposes, for the A @ h GEMM
                hN = []
                for i in range(NCH):
                    pt = psTp.tile([P, D], f16, tag="psT")
                    for j in range(DCH):
                        nc.tensor.transpose(
                            out=pt[:, j * P : (j + 1) * P],
                            in_=hT[j][:, i * P : (i + 1) * P],
                            identity=identity[:],
                        )
                    hni = hnpool.tile([P, D], f16, tag="hN", name=f"hN{l}_{i}")
                    nc.vector.tensor_copy(out=hni[:], in_=pt[:])
                    hN.append(hni)

                # gh_n: runs on hq8 only -> early PE work while hN copies drain
                hb = []
                for j in range(DCH):
                    pghn = psMp.tile([P, N], f32, tag="psM")
                    for k in range(KP):
                        nc.tensor.matmul(
                            out=pghn[:],
                            lhsT=whh8_sb[:, RZCH + j, k, :, :],
                            rhs=hq8[k][:],
                            perf_mode=DR,
                            start=(k == 0),
                            stop=(k == KP - 1),
                        )
                    hbj = gpool.tile([P, N], f16, tag="hb", name=f"hb{l}_{j}")
                    nc.scalar.activation(
                        out=hbj[:], in_=pghn[:], func=Ident,
                        bias=bhhn_sb[:, j : j + 1], scale=SINV,
                    )
                    hb.append(hbj)

                # ah = A @ h  (feature-major out), then fp8/f16 casts
                aq8 = []
                ab16 = []
                for j in range(DCH):
                    pa = psMp.tile([P, N], f32, tag="psM")
                    for i in range(NCH):
                        nc.tensor.matmul(
                            out=pa[:],
                            lhsT=hN[i][:, j * P : (j + 1) * P],
                            rhs=at_sb[:, i, :],
                            start=(i == 0),
                            stop=(i == NCH - 1),
                        )
                    if j % 2 == 0:
                        a8 = apool.tile([P, 2, N], f8, tag="aq8",
                                        name=f"aq8_{l}_{j // 2}", bufs=4)
                        aq8.append(a8)
                    nc.scalar.activation(
                        out=aq8[j // 2][:, j % 2, :], in_=pa[:], func=Ident,
                        bias=0.0, scale=SA,
                    )
                    abj = apool.tile([P, N], f16, tag="ab16", name=f"ab{l}_{j}")
                    nc.vector.tensor_copy(out=abj[:], in_=pa[:])
                    ab16.append(abj)

                # r and z gates: fp8 DoubleRow, gh + gi share one psum group
                rz = []
                for jj in range(RZCH):
                    pg = psRZp.tile([P, N], f32, tag="psRZ")
                    for k in range(KP):
                        nc.tensor.matmul(
                            out=pg[:],
                            lhsT=whh8_sb[:, jj, k, :, :],
                            rhs=hq8[k][:],
                            perf_mode=DR,
                            start=(k == 0),
                            stop=False,
                        )
                    for k in range(KP):
                        nc.tensor.matmul(
                            out=pg[:],
                            lhsT=wc8_l[:, jj, k, :, :],
                            rhs=aq8[k][:],
                            perf_mode=DR,
                            start=False,
                            stop=(k == KP - 1),
                        )
                    tagname = "r" if jj < DCH else "z"
                    g = gpool.tile([P, N], f16, tag=tagname,
                                   name=f"{tagname}{l}_{jj % DCH}")
                    nc.scalar.activation(
                        out=g[:], in_=pg[:], func=Sigmoid,
                        bias=bsum_sb[:, jj : jj + 1], scale=SINV,
                    )
                    rz.append(g)
                r_sb, z_sb = rz[:DCH], rz[DCH:]

                # gi_n: f16 (error-critical), contraction over all 6 chunks
                hnew = []
                hq8n = []
                for j in range(DCH):
                    pgin = psMp.tile([P, N], f32, tag="psM")
                    for k in range(DCH):
                        nc.tensor.matmul(
                            out=pgin[:],
                            lhsT=wcn_l[:, k, j, :],
                            rhs=ab16[k][:],
                            start=(k == 0),
                            stop=(k == DCH - 1),
                        )
                    # n = tanh(gi_n + b_ihn + r * gh_n) ; h' = n + z*(h - n)
                    rn = gpool.tile([P, N], f16, tag="rn")
                    nc.vector.tensor_mul(out=rn[:], in0=r_sb[j][:], in1=hb[j][:])
                    tn = gpool.tile([P, N], f16, tag="tn")
                    nc.vector.tensor_add(out=tn[:], in0=pgin[:], in1=rn[:])
                    nnj = gpool.tile([P, N], f16, tag="nn")
                    nc.scalar.activation(
                        out=nnj[:], in_=tn[:], func=Tanh,
                        bias=bihn_sb[:, j : j + 1],
                    )
                    s_ = gpool.tile([P, N], f16, tag="s_")
                    nc.vector.tensor_sub(out=s_[:], in0=hT[j][:], in1=nnj[:])
                    sz = gpool.tile([P, N], f16, tag="sz")
                    nc.vector.tensor_mul(out=sz[:], in0=z_sb[j][:], in1=s_[:])
                    hj = hpool.tile([P, N], f16, tag="hT", name=f"h{l + 1}_{j}")
                    nc.vector.tensor_add(out=hj[:], in0=nnj[:], in1=sz[:])
                    hnew.append(hj)
                    if l + 1 < L:
                        if j % 2 == 0:
                            h8 = h8pool.tile([P, 2, N], f8, tag="hq8",
                                             name=f"hq8_{l + 1}_{j // 2}")
                            hq8n.append(h8)
                        nc.scalar.activation(
                            out=hq8n[j // 2][:, j % 2, :], in_=hj[:], func=Ident,
                            bias=0.0, scale=SH,
                        )
                hT = hnew
                hq8 = hq8n

            # ---- transpose back to node-major, mask, write out ----
            for i in range(NCH):
                pt = psTp.tile([P, D], f16, tag="psT")
                for j in range(DCH):
                    nc.tensor.transpose(
                        out=pt[:, j * P : (j + 1) * P],
                        in_=hT[j][:, i * P : (i + 1) * P],
                        identity=identity[:],
                    )
                ob = opool.tile([P, D], f32, tag="ob")
                nc.vector.tensor_scalar_mul(
                    out=ob[:], in0=pt[:], scalar1=mask_sb[:, i : i + 1]
                )
                nc.sync.dma_start(out=out[i * P : (i + 1) * P, :], in_=ob[:])

    nc.compile()
    return nc


@functools.lru_cache(maxsize=2)
def _get_nc(pool_wide: bool) -> bass.Bass:
    return build_nc(pool_wide)


def _prep_shared(inputs):
    """Weight tensors identical across graphs, pre-laid-out partition-major."""
    e4 = ml_dtypes.float8_e4m3
    fusion_w = np.ascontiguousarray(
        np.asarray(inputs["fusion_w"], np.float32).astype(np.float16)
    )
    fb = np.asarray(inputs["fusion_b"], np.float32).reshape(DCH, P).T
    fusion_b = np.ascontiguousarray(fb)
    fusion_b64 = np.ascontiguousarray(fb * SH)
    wih_w = np.asarray(inputs["gru_w_ih"], np.float64)   # [K3, D]
    whh_w = np.asarray(inputs["gru_w_hh"], np.float64)
    bih = np.asarray(inputs["gru_b_ih"], np.float32)
    bhh = np.asarray(inputs["gru_b_hh"], np.float32)
    ggnn_w = np.asarray(inputs["ggnn_w"], np.float64)    # [L, D, D]

    # Wc[l] = Wl[l] @ W_ih^T : [L, D, K3]
    wc = np.einsum("lde,fe->ldf", ggnn_w, wih_w)
    # r/z part, fp8 DoubleRow layout [L, P, RZCH, KP, 2, P]
    wc_rz = (wc[:, :, : 2 * D] * SWC).reshape(L, KP, 2, P, RZCH, P)
    wc8 = np.ascontiguousarray(wc_rz.transpose(0, 3, 4, 1, 2, 5)).astype(e4)
    # n part, f16 [L, P, DCH(k), DCH(j), P]
    wc_n = wc[:, :, 2 * D :].reshape(L, DCH, P, DCH, P)
    wcn = np.ascontiguousarray(wc_n.transpose(0, 2, 1, 3, 4)).astype(np.float16)
    # W_hh^T fp8 DoubleRow layout [P, GCH, KP, 2, P]
    whT = (whh_w.T * SWH).reshape(KP, 2, P, GCH, P)
    whh8 = np.ascontiguousarray(whT.transpose(2, 3, 0, 1, 4)).astype(e4)

    bsum = np.ascontiguousarray((bih + bhh).reshape(GCH, P).T)
    bihn = np.ascontiguousarray(bih[2 * D :].reshape(DCH, P).T)
    bhhn = np.ascontiguousarray(bhh[2 * D :].reshape(DCH, P).T)
    word_emb = np.ascontiguousarray(
        np.asarray(inputs["word_emb"], np.float32).astype(np.float16)
    )
    type_table = np.ascontiguousarray(
        np.asarray(inputs["type_table"], np.float32).astype(np.float16)
    )
    return dict(
        word_emb=word_emb, type_table=type_table, fusion_w=fusion_w,
        fusion_b=fusion_b, fusion_b64=fusion_b64, wc8=wc8, wcn=wcn,
        whh8=whh8, bsum=bsum, bihn=bihn, bhhn=bhhn,
    )


def _graph_blockable(inputs, b):
    seg = np.asarray(inputs["token_seg_ids"][b], np.int64)
    tcol = np.arange(T) // P
    return bool(np.all((seg >= tcol * BLK) & (seg < (tcol + 1) * BLK)))


def _prep_graph(inputs, b, pool_wide):
    tok = np.asarray(inputs["node_token_ids"][b], np.int64)
    typ = np.asarray(inputs["node_types"][b], np.int32)
    seg = np.asarray(inputs["token_seg_ids"][b], np.int64)
    lens = np.asarray(inputs["node_token_lens"][b], np.float64)
    glen = int(np.asarray(inputs["graph_node_lens"][b]))
    esrc = np.asarray(inputs["edge_src"][b], np.int64)
    edst = np.asarray(inputs["edge_dst"][b], np.int64)
    ew = np.asarray(inputs["edge_weight"][b], np.float32)

    # token idxs for dma_gather: GS splits of GT idxs, each wrapped into
    # 16 partitions ([p, s] = idx[s*16+p]) and replicated to 128 partitions
    tok16 = tok.astype(np.int16)
    cols = []
    for s in range(GS):
        w16 = tok16[s * GT : (s + 1) * GT].reshape(GT // 16, 16).T  # [16, GT/16]
        cols.append(np.tile(w16, (8, 1)))                           # [128, GT/16]
    tok_idx = np.ascontiguousarray(np.concatenate(cols, axis=1))    # [128, GS*32]

    typ_oh = np.zeros((TYPES, N), np.float16)
    typ_oh[typ, np.arange(N)] = 1.0

    # dense transposed adjacency: AT[src, dst], laid out [P, NCH, N]
    at = np.zeros((N, N), np.float32)
    np.add.at(at, (esrc, edst), ew)
    at = np.ascontiguousarray(
        at.reshape(NCH, P, N).transpose(1, 0, 2)
    ).astype(np.float16)

    # pooling matrix (1/len weights), [P, TCH, BLK or N]
    winv = np.zeros(N, np.float64)
    nzmask = lens != 0
    winv[nzmask] = 1.0 / lens[nzmask]
    tcol = np.arange(T) // P  # token chunk of each token
    if pool_wide:
        poolm = np.zeros((TCH, P, N), np.float32)
        poolm[tcol, np.arange(T) % P, seg] = winv[seg]
    else:
        poolm = np.zeros((TCH, P, BLK), np.float32)
        poolm[tcol, np.arange(T) % P, seg - tcol * BLK] = winv[seg]
    poolm = np.ascontiguousarray(poolm.transpose(1, 0, 2)).astype(np.float16)

    keep = min(glen, MAX_NODE_LEN)
    mask = np.ascontiguousarray(
        (np.arange(N) < keep).astype(np.float32).reshape(NCH, P).T
    )
    return dict(tok_idx=tok_idx, typ_oh=typ_oh, at_w=at, poolm=poolm,
                maskc=mask)


def kernel(**inputs) -> np.ndarray:
    shared = _prep_shared(inputs)
    pool_wide = not all(_graph_blockable(inputs, b) for b in range(B))
    per_graph = [_prep_graph(inputs, b, pool_wide) for b in range(B)]
    nc = _get_nc(pool_wide)
    in_maps = [{**shared, **per_graph[b]} for b in range(B)]
    res = bass_utils.run_bass_kernel_spmd(nc, in_maps, core_ids=list(range(B)))
    global _last_exec_ns
    _last_exec_ns = res.exec_time_ns
    out = np.stack([r["out"] for r in res.results]).astype(np.float32)
    return out


_last_exec_ns = None


# revision 57
# speedup vs baseline: 2.1523x; 1.1196x over previous
# GGNN encoder kernel for Trainium2 (Bass/Tile), data-parallel over the
# batch dimension: 8 graphs -> 8 NeuronCores, one graph per core.
#
# Per-core computation (one graph):
#   type_e  = type_table[node_types]                       # [N, TD]
#   tok_e   = word_emb[node_token_ids]                     # [T, D]   (SWDGE dma_gather)
#   text_e  = segment_mean(tok_e, token_seg_ids)           # [N, D]   (PE matmul w/ pooling matrix)
#   h       = concat(type_e, text_e) @ fusion_w + b        # [N, D]
#   4 x GGNN layer (algebraically refactored):
#     ah   = A @ h               (A dense adjacency; replaces A @ (h@Wl))
#     gi   = ah @ Wc[l]          (Wc[l] = Wl[l] @ W_ih^T, host-precomputed)
#     gh   = h @ W_hh^T
#     GRU(h, gi, gh)
#   out     = mask * h
#
# Precision strategy (validated against the reference numerics):
#   - r/z gate GEMMs and gh_n run in fp8-e4m3 with DoubleRow perf mode
#     (2 MACs/cell/cycle, K=256 per matmul). Operands are pre-scaled by
#     powers of two with matching products (SH*SWH == SA*SWC == 1024) so
#     both GEMMs accumulate into one PSUM group; the 1/1024 descale folds
#     into the activation's scale input.
#   - gi_n (the GRU candidate-gate input transform) is error-critical and
#     stays fp16; A@h, fusion, pooling and embeddings are fp16 as well.
#   - All PSUM accumulation is fp32; GRU elementwise runs in fp16.

import functools

import numpy as np
import ml_dtypes

import concourse.bass as bass
import concourse.mybir as mybir
import concourse.tile as tile
from concourse import bacc, bass_utils
from concourse.masks import make_identity

# Problem shapes (hardcoded: kernel must be self-contained).
B, N, T, D, TD, L = 8, 512, 2048, 768, 128, 4
V, TYPES = 30522, 64
MAX_NODE_LEN = 512
K3 = 3 * D            # 2304 stacked GRU gate rows
F = TD + D            # 896 fused embedding dim
P = 128               # partitions
NCH = N // P          # 4 node chunks
TCH = T // P          # 16 token chunks
DCH = D // P          # 6 feature chunks
FCH = F // P          # 7 fused-dim chunks
GCH = 3 * DCH         # 18 gate row chunks
RZCH = 2 * DCH        # 12 r/z gate row chunks
KP = DCH // 2         # 3 contraction k-pairs for DoubleRow
BLK = N // TCH        # 32 nodes per token chunk (block-pooling case)
GS = 2                # token gather splits
GT = T // GS          # tokens per gather split (512)
GC = GT // P          # 128-chunks per gather split (4)

SH = 64.0             # fp8 scale on h
SWH = 16.0            # fp8 scale on W_hh^T
SA = 16.0             # fp8 scale on ah
SWC = 64.0            # fp8 scale on Wc_rz
SINV = 1.0 / (SH * SWH)   # descale (== 1/(SA*SWC))

f32 = mybir.dt.float32
f16 = mybir.dt.float16
f8 = mybir.dt.float8e4
MULT = mybir.AluOpType.mult
ADD = mybir.AluOpType.add
BYPASS = mybir.AluOpType.bypass
MULT = mybir.AluOpType.mult
ADD = mybir.AluOpType.add
BYPASS = mybir.AluOpType.bypass
i16 = mybir.dt.int16
DR = mybir.MatmulPerfMode.DoubleRow

Sigmoid = mybir.ActivationFunctionType.Sigmoid
Tanh = mybir.ActivationFunctionType.Tanh
Ident = mybir.ActivationFunctionType.Identity


def build_nc(pool_wide: bool, zero_bias: bool = True) -> bass.Bass:
    nc = bacc.Bacc(num_swdge_queues=2, dynamic_dma_scratch_size=32768)

    # All host-side tensors are pre-laid-out partition-major so every DMA is
    # contiguous per partition.
    tok_idx = nc.dram_tensor("tok_idx", [P, GS * (GT // 16)], i16,
                             kind="ExternalInput")  # [128, 4*32] wrapped idxs
    typ_oh = nc.dram_tensor("typ_oh", [TYPES, N], f16, kind="ExternalInput")
    word_emb = nc.dram_tensor("word_emb", [V, D], f8, kind="ExternalInput")
    type_table = nc.dram_tensor("type_table", [TYPES, TD], f16, kind="ExternalInput")
    pool_w = N if pool_wide else BLK
    poolm = nc.dram_tensor("poolm", [P, TCH, pool_w], f8, kind="ExternalInput")
    at_w = nc.dram_tensor("at_w", [P, NCH, N], f16, kind="ExternalInput")
    fusion_w = nc.dram_tensor("fusion_w", [F, D], f16, kind="ExternalInput")
    fusion_b = nc.dram_tensor("fusion_b", [P, DCH], f32, kind="ExternalInput")
    wc8 = nc.dram_tensor("wc8", [L, P, RZCH, KP, 2, P], f8, kind="ExternalInput")
    wcn = nc.dram_tensor("wcn", [L, P, DCH, DCH, P], f16, kind="ExternalInput")
    whh8 = nc.dram_tensor("whh8", [P, GCH, KP, 2, P], f8, kind="ExternalInput")
    bsum = nc.dram_tensor("bsum", [P, GCH], f32, kind="ExternalInput")
    bihn = nc.dram_tensor("bihn", [P, DCH], f32, kind="ExternalInput")
    bhhn = nc.dram_tensor("bhhn", [P, DCH], f32, kind="ExternalInput")
    nmaskf = nc.dram_tensor("nmaskf", [P, N], f16, kind="ExternalInput")
    # out[p, i, j, m] = h_final[i*128 + p, j*128 + m] (host re-assembles)
    out = nc.dram_tensor("out", [P, NCH, DCH, P], f32, kind="ExternalOutput")

    with tile.TileContext(nc) as tc:
        with (
            tc.tile_pool(name="consts", bufs=1) as consts,
            tc.tile_pool(name="wres", bufs=1) as wres,
            tc.tile_pool(name="wstr", bufs=2) as wstr,
            tc.tile_pool(name="hpool", bufs=12) as hpool,
            tc.tile_pool(name="h8pool", bufs=6) as h8pool,
            tc.tile_pool(name="hnpool", bufs=2) as hnpool,
            tc.tile_pool(name="apool", bufs=8) as apool,
            tc.tile_pool(name="gpool", bufs=7) as gpool,
            tc.tile_pool(name="opool", bufs=2) as opool,
            tc.tile_pool(name="psT", bufs=3, space="PSUM") as psTp,
            tc.tile_pool(name="psRZ", bufs=3, space="PSUM") as psRZp,
            tc.tile_pool(name="psM", bufs=2, space="PSUM") as psMp,
        ):
            # ---- token gather first: it gates the whole front of the kernel
            tok_idx_sb = consts.tile([P, T // 16], i16)
            nc.sync.dma_start(out=tok_idx_sb[:], in_=tok_idx[:])
            pool_sb = consts.tile([P, TCH, pool_w], f8)
            nc.sync.dma_start(out=pool_sb[:], in_=poolm[:])

            # type embeddings via one-hot matmul: two tiny DMAs + one PE op
            tt_sb = consts.tile([TYPES, TD], f16)
            nc.scalar.dma_start(out=tt_sb[:], in_=type_table[:])
            oh_sb = consts.tile([TYPES, N], f16)
            nc.scalar.dma_start(out=oh_sb[:], in_=typ_oh[:])

            gath = []
            for s in range(GS):
                tg = wstr.tile([P, GC, D], f8, tag="wc8", name=f"tokg{s}")
                nc.gpsimd.dma_gather(
                    tg[:],
                    word_emb[:],
                    tok_idx_sb[:, s * (GT // 16) : (s + 1) * (GT // 16)],
                    GT,
                    GT,
                    D,
                    queue_num=s % 2,
                )
                gath.append(tg)

            # ---- remaining constants / small inputs ----
            identity = consts.tile([P, P], f16)
            make_identity(nc, identity[:])
            bsum_sb = consts.tile([P, GCH], f32)
            nc.sync.dma_start(out=bsum_sb[:], in_=bsum[:])
            bihn_sb = consts.tile([P, DCH], f32)
            nc.sync.dma_start(out=bihn_sb[:], in_=bihn[:])
            bhhn_sb = consts.tile([P, DCH], f32)
            nc.sync.dma_start(out=bhhn_sb[:], in_=bhhn[:])
            fb_sb = consts.tile([P, DCH], f32)
            nc.sync.dma_start(out=fb_sb[:], in_=fusion_b[:])
            nmask_sb = consts.tile([P, N], f16)
            nc.sync.dma_start(out=nmask_sb[:], in_=nmaskf[:])
            # queue spacer: delays the big weight transfers on the sync DGE
            # just enough that the token-gather transfers keep the DMA
            # engines during the front (measurably faster end-to-end)
            spacer_sb = consts.tile([P, DCH], f32, name="spacer")
            nc.sync.dma_start(out=spacer_sb[:], in_=bihn[:])

            # fusion weights (needed ~6us in)
            fw = []
            for k in range(FCH):
                fwk = consts.tile([P, D], f16, name=f"fw{k}")
                nc.scalar.dma_start(out=fwk[:], in_=fusion_w[k * P : (k + 1) * P, :])
                fw.append(fwk)

            # adjacency + resident GRU hidden weights; clock-stamped so
            # these big transfers don't cut ahead of the token gathers on
            # the shared DMA engines during the front
            at_sb = wres.tile([P, NCH, N], f16)
            with tc.tile_wait_until(0.100):
                nc.sync.dma_start(out=at_sb[:], in_=at_w[:])
            whh8_sb = wres.tile([P, GCH, KP, 2, P], f8)
            with tc.tile_wait_until(0.011):
                nc.sync.dma_start(out=whh8_sb[:], in_=whh8[:])

            # layer-0 streamed weights
            def load_layer_w(l, wait=None):
                wc8_l = wstr.tile([P, RZCH, KP, 2, P], f8, tag="wc8",
                                  name=f"wc8_{l}")
                with tc.tile_wait_until(wait or 0, enable=wait is not None):
                    nc.sync.dma_start(out=wc8_l[:], in_=wc8[l])
                wcn_l = wstr.tile([P, DCH, DCH, P], f16, tag="wcn",
                                  name=f"wcn_{l}")
                with tc.tile_wait_until((wait or 0) + 0.003,
                                        enable=wait is not None):
                    nc.sync.dma_start(out=wcn_l[:], in_=wcn[l])
                return wc8_l, wcn_l

            next_w = load_layer_w(0, wait=0.016)

            # ---- fused embedding (feature-major [f, n]), single f16 tile ----
            fusedT = consts.tile([P, FCH, N], f16)

            # type_eT = type_table.T @ onehot  (one matmul, K=64)
            ptyp = psMp.tile([P, N], f32, tag="psM")
            nc.tensor.matmul(
                out=ptyp[:], lhsT=tt_sb[:], rhs=oh_sb[:], start=True, stop=True
            )
            nc.vector.tensor_copy(out=fusedT[:, 0, :], in_=ptyp[:])

            # token pooling: PE matmul pools 128 tokens -> 32 nodes and
            # transposes to feature-major in one pass
            for s in range(GS):
                tg = gath[s]
                for c2 in range(GC):
                    c = s * GC + c2
                    if pool_wide:
                        for fi in range(DCH):
                            pc = psMp.tile([P, N], f32, tag="psM")
                            nc.tensor.matmul(
                                out=pc[:],
                                lhsT=tg[:, c2, fi * P : (fi + 1) * P],
                                rhs=pool_sb[:, c, :],
                                start=True,
                                stop=True,
                            )
                            if c == 0:
                                nc.vector.tensor_copy(
                                    out=fusedT[:, 1 + fi, :], in_=pc[:]
                                )
                            else:
                                nc.vector.tensor_add(
                                    out=fusedT[:, 1 + fi, :],
                                    in0=fusedT[:, 1 + fi, :],
                                    in1=pc[:],
                                )
                    else:
                        pc = psMp.tile([P, DCH, BLK], f32, tag="psM")
                        for fi in range(DCH):
                            nc.tensor.matmul(
                                out=pc[:, fi, :],
                                lhsT=tg[:, c2, fi * P : (fi + 1) * P],
                                rhs=pool_sb[:, c, :],
                                start=True,
                                stop=True,
                            )
                        nc.vector.tensor_copy(
                            out=fusedT[:, 1 : 1 + DCH, c * BLK : (c + 1) * BLK],
                            in_=pc[:],
                        )

            # ---- fusion matmul -> hT0 (f16) + hq8_0, two node-halves so
            # half 0 starts as soon as gather split 0 is pooled ----
            NH = N // 2
            hT = [hpool.tile([P, N], f16, tag="hT", name=f"h0_{j}")
                  for j in range(DCH)]
            hq8 = [h8pool.tile([P, 2, N], f8, tag="hq8", name=f"hq8_0_{k}")
                   for k in range(KP)]
            for hh in range(2):
                cols = slice(hh * NH, (hh + 1) * NH)
                for jg in range(DCH // 2):
                    pf = psMp.tile([P, 2, NH], f32, tag="psM")
                    for j2 in range(2):
                        j = jg * 2 + j2
                        for k in range(FCH):
                            nc.tensor.matmul(
                                out=pf[:, j2, :],
                                lhsT=fw[k][:, j * P : (j + 1) * P],
                                rhs=fusedT[:, k, cols],
                                start=(k == 0),
                                stop=(k == FCH - 1),
                            )
                    for j2 in range(2):
                        j = jg * 2 + j2
                        nc.scalar.activation(
                            out=hT[j][:, cols], in_=pf[:, j2, :], func=Ident,
                            bias=fb_sb[:, j : j + 1],
                        )
                        nc.vector.tensor_scalar(
                            out=hq8[j // 2][:, j % 2, cols], in0=pf[:, j2, :],
                            scalar1=fb_sb[:, j : j + 1], scalar2=SH,
                            op0=ADD, op1=MULT,
                        )

            # ---- GGNN layers ----
            for l in range(L):
                last = l == L - 1
                wc8_l, wcn_l = next_w
                if l + 1 < L:
                    next_w = load_layer_w(l + 1, 0)

                # node-major h (f16) via PE transposes, for the A @ h GEMM.
                # j-major emission: the first 5*NCH blocks only need hT[0..4],
                # so PE keeps running while the last ew chunk of the previous
                # layer drains; only the final NCH blocks wait on hT[5].
                pts = [psTp.tile([P, D], f16, tag="psT", name=f"pt{l}_{i}")
                       for i in range(NCH)]
                for j in range(DCH):
                    for i in range(NCH):
                        nc.tensor.transpose(
                            out=pts[i][:, j * P : (j + 1) * P],
                            in_=hT[j][:, i * P : (i + 1) * P],
                            identity=identity[:],
                        )
                hN = []
                for i in range(NCH):
                    hni = hnpool.tile([P, D], f16, tag="hN", name=f"hN{l}_{i}")
                    nc.vector.tensor_copy(out=hni[:], in_=pts[i][:])
                    hN.append(hni)

                # ah = A @ h  (feature-major out), then fp8/f16 casts
                aq8 = []
                ab16 = []
                for j in range(DCH):
                    pa = psMp.tile([P, N], f32, tag="psM")
                    for i in range(NCH):
                        nc.tensor.matmul(
                            out=pa[:],
                            lhsT=hNall[:, i, j * P : (j + 1) * P],
                            rhs=at_sb[:, i, :],
                            start=(i == 0),
                            stop=(i == NCH - 1),
                        )
                    if j % 2 == 0:
                        a8 = apool.tile([P, 2, N], f8, tag="aq8",
                                        name=f"aq8_{l}_{j // 2}", bufs=4)
                        aq8.append(a8)
                    nc.scalar.activation(
                        out=aq8[j // 2][:, j % 2, :], in_=pa[:], func=Ident,
                        bias=0.0, scale=SA,
                    )
                    abj = apool.tile([P, N], f16, tag="ab16", name=f"ab{l}_{j}")
                    nc.vector.tensor_copy(out=abj[:], in_=pa[:])
                    ab16.append(abj)

                # r and z gates: fp8 DoubleRow, gh + gi share one psum
                # group; software-pipelined so a group's gh half runs as soon
                # as a psum slot frees, hiding the aq8 cast latency
                def gh_half(pg, jj):
                    for k in range(KP):
                        nc.tensor.matmul(
                            out=pg[:],
                            lhsT=whh8_sb[:, jj, k, :, :],
                            rhs=hq8[k][:],
                            perf_mode=DR,
                            start=(k == 0),
                            stop=False,
                        )

                rz = []
                for jj in range(RZCH):
                    if jj == 0:
                        rz_ps = []
                        for j0 in range(3):
                            pg0 = psRZp.tile([P, N], f32, tag="psRZ",
                                             name=f"prz{l}_{j0}")
                            gh_half(pg0, j0)
                            rz_ps.append(pg0)
                    pg = rz_ps[jj]
                    for k in range(KP):
                        nc.tensor.matmul(
                            out=pg[:],
                            lhsT=wc8_l[:, jj, k, :, :],
                            rhs=aq8[k][:],
                            perf_mode=DR,
                            start=False,
                            stop=(k == KP - 1),
                        )
                    tagname = "r" if jj < DCH else "z"
                    g = gpool.tile([P, N], f16, tag=tagname,
                                   name=f"{tagname}{l}_{jj % DCH}")
                    nc.scalar.activation(
                        out=g[:], in_=pg[:], func=Sigmoid,
                        bias=0.0 if zero_bias else bsum_sb[:, jj : jj + 1],
                        scale=SINV,
                    )
                    rz.append(g)
                    if jj + 3 < RZCH:
                        png = psRZp.tile([P, N], f32, tag="psRZ",
                                         name=f"prz{l}_{jj + 3}")
                        gh_half(png, jj + 3)
                        rz_ps.append(png)
                r_sb, z_sb = rz[:DCH], rz[DCH:]

                # gh_n: fp8 on hq8, psums on the now-free psRZ ring; with
                # zero biases  r * gh_n  comes straight off PSUM in one
                # fused DVE op (no ACT hb pass)
                ghn_ps = []
                for j in range(DCH):
                    pghn = psRZp.tile([P, N], f32, tag="psRZ",
                                      name=f"pghn{l}_{j}")
                    for k in range(KP):
                        nc.tensor.matmul(
                            out=pghn[:],
                            lhsT=whh8_sb[:, RZCH + j, k, :, :],
                            rhs=hq8[k][:],
                            perf_mode=DR,
                            start=(k == 0),
                            stop=(k == KP - 1),
                        )
                    ghn_ps.append(pghn)
                    if not zero_bias:
                        hbj = gpool.tile([P, N], f16, tag="hb",
                                         name=f"hb{l}_{j}")
                        nc.scalar.activation(
                            out=hbj[:], in_=pghn[:], func=Ident,
                            bias=bhhn_sb[:, j : j + 1], scale=SINV,
                        )
                        hb.append(hbj)

                # gi_n: f16 (error-critical), contraction over all 6 chunks
                hnew = []
                hq8n = []
                for j in range(DCH):
                    pgin = psMp.tile([P, N], f32, tag="psM")
                    for k in range(DCH):
                        nc.tensor.matmul(
                            out=pgin[:],
                            lhsT=wcn_l[:, k, j, :],
                            rhs=ab16[k][:],
                            start=(k == 0),
                            stop=(k == DCH - 1),
                        )
                    # n = tanh(gi_n + b_ihn + r * gh_n) ; h' = n + z*(h - n)
                    rn = gpool.tile([P, N], f16, tag="rn")
                    if zero_bias:
                        nc.vector.scalar_tensor_tensor(
                            out=rn[:], in0=ghn_ps[j][:], scalar=SINV,
                            in1=r_sb[j][:], op0=MULT, op1=MULT,
                        )
                    else:
                        nc.vector.tensor_mul(out=rn[:], in0=r_sb[j][:],
                                             in1=hb[j][:])
                    tn = gpool.tile([P, N], f16, tag="tn")
                    nc.vector.tensor_add(out=tn[:], in0=pgin[:], in1=rn[:])
                    nnj = gpool.tile([P, N], f16, tag="nn")
                    nc.scalar.activation(
                        out=nnj[:], in_=tn[:], func=Tanh,
                        bias=0.0 if zero_bias else bihn_sb[:, j : j + 1],
                    )
                    s_ = gpool.tile([P, N], f16, tag="s_")
                    nc.vector.tensor_sub(out=s_[:], in0=hT[j // 2][:, j % 2, :], in1=nnj[:])
                    sz = gpool.tile([P, N], f16, tag="sz")
                    nc.vector.tensor_mul(out=sz[:], in0=z_sb[j][:], in1=s_[:])
                    hj = hpool.tile([P, N], f16, tag="hT", name=f"h{l + 1}_{j}")
                    nc.vector.tensor_add(out=hj[:], in0=nnj[:], in1=sz[:])
                    hnew.append(hj)
                    if not last:
                        if j % 2 == 0:
                            h8 = h8pool.tile([P, 2, N], f8, tag="hq8",
                                             name=f"hq8_{l + 1}_{j // 2}")
                            hq8n.append(h8)
                        nc.gpsimd.tensor_scalar_mul(
                            out=hq8n[j // 2][:, j % 2, :], in0=hj[:], scalar1=SH
                        )
                    else:
                        # mask, transpose to fp32 psum blocks (regular matmul
                        # with the f16 identity), copy via ACT, DMA per j
                        if full_mask:
                            hm = hj
                        else:
                            hmx = gpool.tile([P, N], f16, tag="hm")
                            nc.vector.tensor_mul(out=hmx[:], in0=hj,
                                                 in1=nmask_sb[:])
                            hm = hmx[:]
                        po = psRZp.tile([P, NCH, P], f32, tag="psRZ",
                                        name=f"pout{j}")
                        for i in range(NCH):
                            nc.tensor.matmul(
                                out=po[:, i, :],
                                lhsT=hm[:, i * P : (i + 1) * P],
                                rhs=identity[:],
                                start=True,
                                stop=True,
                            )
                        ob = opool.tile([P, NCH, P], f32, tag="ob",
                                        name=f"ob{j}", bufs=4)
                        nc.scalar.copy(out=ob[:], in_=po[:])
                        eng = (nc.scalar, nc.gpsimd, nc.sync)[j % 3]
                        eng.dma_start(out=out[:, :, j, :], in_=ob[:])
                hT = hnew
                hq8 = hq8n

    nc.compile()
    return nc


@functools.lru_cache(maxsize=4)
def _get_nc(pool_wide: bool, zero_bias: bool = True) -> bass.Bass:
    return build_nc(pool_wide, zero_bias)


def _prep_shared(inputs):
    """Weight tensors identical across graphs, pre-laid-out partition-major."""
    e4 = ml_dtypes.float8_e4m3
    fusion_w = np.ascontiguousarray(
        np.asarray(inputs["fusion_w"], np.float32).astype(np.float16)
    )
    fusion_b = np.ascontiguousarray(
        np.asarray(inputs["fusion_b"], np.float32).reshape(DCH, P).T
    )
    wih_w = np.asarray(inputs["gru_w_ih"], np.float64)   # [K3, D]
    whh_w = np.asarray(inputs["gru_w_hh"], np.float64)
    bih = np.asarray(inputs["gru_b_ih"], np.float32)
    bhh = np.asarray(inputs["gru_b_hh"], np.float32)
    ggnn_w = np.asarray(inputs["ggnn_w"], np.float64)    # [L, D, D]

    # Wc[l] = Wl[l] @ W_ih^T : [L, D, K3]
    wc = np.einsum("lde,fe->ldf", ggnn_w, wih_w)
    # r/z part, fp8 DoubleRow layout [L, P, RZCH, KP, 2, P]
    wc_rz = (wc[:, :, : 2 * D] * SWC).reshape(L, KP, 2, P, RZCH, P)
    wc8 = np.ascontiguousarray(wc_rz.transpose(0, 3, 4, 1, 2, 5)).astype(e4)
    # n part, f16 [L, P, DCH(k), DCH(j), P]
    wc_n = wc[:, :, 2 * D :].reshape(L, DCH, P, DCH, P)
    wcn = np.ascontiguousarray(wc_n.transpose(0, 2, 1, 3, 4)).astype(np.float16)
    # W_hh^T fp8 DoubleRow layout [P, GCH, KP, 2, P]
    whT = (whh_w.T * SWH).reshape(KP, 2, P, GCH, P)
    whh8 = np.ascontiguousarray(whT.transpose(2, 3, 0, 1, 4)).astype(e4)

    bsum = np.ascontiguousarray((bih + bhh).reshape(GCH, P).T)
    bihn = np.ascontiguousarray(bih[2 * D :].reshape(DCH, P).T)
    bhhn = np.ascontiguousarray(bhh[2 * D :].reshape(DCH, P).T)
    word_emb = np.ascontiguousarray(
        (np.asarray(inputs["word_emb"], np.float32) * 32).astype(e4)
    )
    type_table = np.ascontiguousarray(
        np.asarray(inputs["type_table"], np.float32).astype(np.float16)
    )
    return dict(
        word_emb=word_emb, type_table=type_table, fusion_w=fusion_w,
        fusion_b=fusion_b, wc8=wc8, wcn=wcn, whh8=whh8, bsum=bsum,
        bihn=bihn, bhhn=bhhn,
    )


def _graph_blockable(inputs, b):
    seg = np.asarray(inputs["token_seg_ids"][b], np.int64)
    tcol = np.arange(T) // P
    return bool(np.all((seg >= tcol * BLK) & (seg < (tcol + 1) * BLK)))


def _prep_graph(inputs, b, pool_wide):
    tok = np.asarray(inputs["node_token_ids"][b], np.int64)
    typ = np.asarray(inputs["node_types"][b], np.int32)
    seg = np.asarray(inputs["token_seg_ids"][b], np.int64)
    lens = np.asarray(inputs["node_token_lens"][b], np.float64)
    glen = int(np.asarray(inputs["graph_node_lens"][b]))
    esrc = np.asarray(inputs["edge_src"][b], np.int64)
    edst = np.asarray(inputs["edge_dst"][b], np.int64)
    ew = np.asarray(inputs["edge_weight"][b], np.float32)

    # token idxs for dma_gather: GS splits of GT idxs, each wrapped into
    # 16 partitions ([p, s] = idx[s*16+p]) and replicated to 128 partitions
    tok16 = tok.astype(np.int16)
    cols = []
    for s in range(GS):
        w16 = tok16[s * GT : (s + 1) * GT].reshape(GT // 16, 16).T  # [16, GT/16]
        cols.append(np.tile(w16, (8, 1)))                           # [128, GT/16]
    tok_idx = np.ascontiguousarray(np.concatenate(cols, axis=1))    # [128, GS*32]

    typ_oh = np.zeros((TYPES, N), np.float16)
    typ_oh[typ, np.arange(N)] = 1.0

    # dense transposed adjacency: AT[src, dst], laid out [P, NCH, N]
    at = np.zeros((N, N), np.float32)
    np.add.at(at, (esrc, edst), ew)
    at = np.ascontiguousarray(
        at.reshape(NCH, P, N).transpose(1, 0, 2)
    ).astype(np.float16)

    # pooling matrix (1/len weights), [P, TCH, BLK or N]
    winv = np.zeros(N, np.float64)
    nzmask = lens != 0
    winv[nzmask] = 1.0 / lens[nzmask]
    tcol = np.arange(T) // P  # token chunk of each token
    if pool_wide:
        poolm = np.zeros((TCH, P, N), np.float32)
        poolm[tcol, np.arange(T) % P, seg] = winv[seg]
    else:
        poolm = np.zeros((TCH, P, BLK), np.float32)
        poolm[tcol, np.arange(T) % P, seg - tcol * BLK] = winv[seg]
    poolm = np.ascontiguousarray(poolm.transpose(1, 0, 2) / 32).astype(
        ml_dtypes.float8_e4m3
    )

    keep = min(glen, MAX_NODE_LEN)
    nmask = np.broadcast_to(
        (np.arange(N) < keep).astype(np.float16)[None, :], (P, N)
    )
    return dict(tok_idx=tok_idx, typ_oh=typ_oh, at_w=at, poolm=poolm,
                nmaskf=np.ascontiguousarray(nmask))


def kernel(**inputs) -> np.ndarray:
    shared = _prep_shared(inputs)
    pool_wide = not all(_graph_blockable(inputs, b) for b in range(B))
    per_graph = [_prep_graph(inputs, b, pool_wide) for b in range(B)]
    zb = all(
        not np.any(np.asarray(inputs[k]))
        for k in ("fusion_b", "gru_b_ih", "gru_b_hh")
    )
    nc = _get_nc(pool_wide, zb)
    in_maps = [{**shared, **per_graph[b]} for b in range(B)]
    res = bass_utils.run_bass_kernel_spmd(nc, in_maps, core_ids=list(range(B)))
    global _last_exec_ns
    _last_exec_ns = res.exec_time_ns
    # out[p, i, j, m] -> h[i*128+p, j*128+m]
    out = np.stack(
        [
            np.asarray(r["out"], np.float32)
            .transpose(1, 0, 2, 3)
            .reshape(N, D)
            for r in res.results
        ]
    )
    return out


_last_exec_ns = None
